# revision 3
# baseline (speedup 1.0000x reference)
"""Variant of kernel.py without cross-device collectives: two-phase BN.

Phase 1 (sharded, 8 cores): compute pre-BN activations + per-shard
sum/sumsq. Host reduces stats. Phase 2 (sharded): apply BN + relu.
"""

import numpy as np
import jax
import jax.numpy as jnp
from jax.sharding import Mesh, PartitionSpec as P
from jax.experimental.shard_map import shard_map

B, CIN, COUT, H, W = 2, 64, 64, 64, 2048
EPS = 1e-5
NCORES = 8
SLABS = 4
SH = H // SLABS

_phase1 = None
_phase2 = None


def _body1(xs, mask_s, w_spatial, b_spatial, w_channel, b_channel, w_agg):
    b, c, hp, wp = xs.shape
    h, w = hp - 2, wp - 2
    xun = jnp.stack([xs[:, :, di:di + h, dj:dj + w]
                     for di in range(3) for dj in range(3)], axis=2)  # [1,67,9,h,w]
    mun = jnp.stack([mask_s[:, 0, di:di + h, dj:dj + w]
                     for di in range(3) for dj in range(3)], axis=1)  # [1,9,h,w]
    pos = xun[:, :3]
    feat = xun[:, 3:]
    rng = jnp.sqrt(jnp.sum(pos * pos, axis=1, keepdims=True))
    pn4 = jnp.concatenate([pos, rng], axis=1)
    pn_p0 = pn4 - pn4[:, :, 4:5]
    ws = jnp.einsum('bmkhw,cm->bckhw', pn_p0, w_spatial) + b_spatial[None, :, None, None, None]
    wc = jnp.einsum('bmkhw,cm->bckhw', pn_p0, w_channel) + b_channel[None, :, None, None, None]
    ws = jnp.max(ws, axis=1, keepdims=True)
    wc = jnp.max(wc, axis=2, keepdims=True)
    ws = jax.nn.softmax(ws, axis=2)
    wc = jax.nn.softmax(wc, axis=1)
    weights = (ws + wc) * mun[:, None]
    wf = (weights * feat).transpose(0, 2, 1, 3, 4).reshape(1, 9 * CIN, h * w)
    out = jnp.einsum('of,bfn->bon', w_agg.astype(jnp.bfloat16),
                     wf.astype(jnp.bfloat16),
                     preferred_element_type=jnp.float32)  # [1, COUT, h*w]
    s1 = jnp.sum(out, axis=(0, 2))
    s2 = jnp.sum(out * out, axis=(0, 2))
    return out.reshape(1, COUT, h, w), s1[None], s2[None]


def _body2(o, a, bvec):
    # o: [1, COUT, SH, W]; a, b: [COUT] replicated
    return jax.nn.relu(o * a[None, :, None, None] + bvec[None, :, None, None])


def _build():
    devs = jax.devices()[:NCORES]
    mesh = Mesh(np.asarray(devs), ('core',))
    p1 = jax.jit(shard_map(
        _body1, mesh=mesh,
        in_specs=(P('core'), P('core'), P(), P(), P(), P(), P()),
        out_specs=(P('core'), P('core'), P('core')),
        check_rep=False,
    ))
    p2 = jax.jit(shard_map(
        _body2, mesh=mesh,
        in_specs=(P('core'), P(), P()),
        out_specs=P('core'),
        check_rep=False,
    ))
    return p1, p2


def kernel(x, mask, w_spatial, b_spatial, w_channel, b_channel, w_agg,
           gamma, beta):
    global _phase1, _phase2
    x = np.asarray(x, np.float32)
    mask_f = np.asarray(mask).astype(np.float32)

    xp = np.pad(x, ((0, 0), (0, 0), (1, 1), (1, 1)))
    mp = np.pad(mask_f, ((0, 0), (0, 0), (1, 1), (1, 1)))
    xs = np.concatenate([xp[b:b + 1, :, s * SH:s * SH + SH + 2, :]
                         for b in range(B) for s in range(SLABS)], axis=0)
    ms = np.concatenate([mp[b:b + 1, :, s * SH:s * SH + SH + 2, :]
                         for b in range(B) for s in range(SLABS)], axis=0)

    if _phase1 is None:
        _phase1, _phase2 = _build()

    o, s1, s2 = _phase1(jnp.asarray(xs), jnp.asarray(ms),
                        jnp.asarray(w_spatial), jnp.asarray(b_spatial),
                        jnp.asarray(w_channel), jnp.asarray(b_channel),
                        jnp.asarray(w_agg))
    s1 = np.asarray(s1).sum(0)
    s2 = np.asarray(s2).sum(0)
    cnt = float(B * H * W)
    mu = s1 / cnt
    var = s2 / cnt - mu * mu
    g = np.asarray(w_agg.dtype.type(0))  # noqa - placeholder no-op
    gamma = np.asarray(gamma)
    beta = np.asarray(beta)
    a = gamma / np.sqrt(var + EPS)
    bvec = beta - mu * a

    out_sh = _phase2(o, jnp.asarray(a.astype(np.float32)),
                     jnp.asarray(bvec.astype(np.float32)))
    out_sh = np.asarray(out_sh)

    out = np.empty((B, COUT, H, W), np.float32)
    i = 0
    for b in range(B):
        for s in range(SLABS):
            out[b, :, s * SH:(s + 1) * SH, :] = out_sh[i]
            i += 1
    return out
